# revision 19
# baseline (speedup 1.0000x reference)
import os
import numpy as np

# nn_PixelflyLinear: y = (x @ w1.T) @ w2.T + b + butterfly_matmul(x, weight, flat_idx)
# Data-parallel over tokens: 8 cores x 512 tokens, weights replicated.
# Device computes yT (out_f on partitions, tokens on free dim); host transposes.

TOKENS, IN_F, OUT_F, LOWRANK = 4096, 4096, 4096, 256
BLOCK, ACTIVE, NB = 256, 5, 16
NCORES = 8
TPC = TOKENS // NCORES          # 512 tokens per core
NG = OUT_F // 128               # 32 output half-block groups
NXT = IN_F // 128               # 32 input tiles
NSLOT = 12                      # logical lhsT slots per group (10 bf + 2 lowrank)
K8 = 3                          # butterfly j-slots per group in fp8 DoubleRow
NS16 = 2 * (ACTIVE - K8) + 2    # fp16 slots per group: bf fp16 + 2 w2 slots
PSCALE = 4096.0                 # psum scale: fp16 W x4096; fp8 (Wx256)*(xx16)
NWARM = 7                       # dummy matmuls to trip HAM to 8/8 before real work

_CACHE = {}
LAST = {"exec_time_ns": None}


def _derive_xtile_idx(flat):
    xtile_idx = np.zeros((NG, 10), np.int64)
    for ob in range(NB):
        for j in range(ACTIVE):
            m = int(flat[ob, j])
            q = m // ACTIVE
            for rh in range(2):
                for kh in range(2):
                    xtile_idx[ob * 2 + rh, j * 2 + kh] = q * 2 + kh
    return xtile_idx


def _build(xtile_idx):
    import concourse.bacc as bacc
    import concourse.mybir as mybir
    import concourse.tile as tile

    nc = bacc.Bacc("TRN2", target_bir_lowering=False, debug=False,
                   num_devices=NCORES)
    dt = mybir.dt

    LEADS = 6
    # x-tile chunks (tapered: small first for early PE start, fat later)
    XCH = [(0, 1), (1, 4), (4, 10), (10, 18), (18, 26), (26, 32)]
    # w1 slot ranges per DMA piece (slot = i*2+lh, 64 slots total)
    W1CH = [(0, 4), (4, 12), (12, 32), (32, 64)]
    # y-out group chunks (tapered at the end to shrink the drain tail)
    YCH = [(0, 4), (4, 8), (8, 12), (12, 16), (16, 20), (20, 24), (24, 28),
           (28, 30), (30, 31), (31, 32)]

    xpack_d = nc.dram_tensor("xpack", [128, NXT * TPC], dt.float16,
                             kind="ExternalInput")
    x8_d = nc.dram_tensor("x8pack", [128, NXT * TPC], dt.float8e4,
                          kind="ExternalInput")
    w1_d = nc.dram_tensor("w1pack", [128, 64 * 128], dt.float16,
                          kind="ExternalInput")
    g_d = nc.dram_tensor("gpack", [NG // 2, 128, 2 * NS16 * 128], dt.float16,
                         kind="ExternalInput")
    g8_d = nc.dram_tensor("g8pack", [NG // 2, 128, 2 * K8 * 2 * 128],
                          dt.float8e4, kind="ExternalInput")
    b_d = nc.dram_tensor("bpack", [128, NG], dt.float32, kind="ExternalInput")
    y_d = nc.dram_tensor("y", [128, NG * TPC], dt.float16,
                         kind="ExternalOutput")

    with tile.TileContext(nc) as tc:
        with (
            tc.tile_pool(name="res", bufs=1) as res_pool,
            tc.tile_pool(name="upsum", bufs=1, space="PSUM") as upsum,
            tc.tile_pool(name="gpsum", bufs=6, space="PSUM") as gpsum,
        ):
            X8CH = [(0, 4), (4, 8), (8, 12), (12, 16)]  # block units
            xch = [None] * len(XCH)          # SBUF chunk tiles
            x8ch = [None] * len(X8CH)
            w1p = [None] * len(W1CH)
            gpt = [None] * (NG // 2)         # fp16 gpack pair tiles
            g8t = [None] * (NG // 2)         # fp8 DR pair tiles
            accs = [None] * NG

            def dma_x(j):
                lo, hi = XCH[j]
                t = res_pool.tile([128, (hi - lo) * TPC], dt.float16,
                                  tag=f"xc{j}", name=f"xc{j}")
                nc.scalar.dma_start(t[:], xpack_d[:, lo * TPC:hi * TPC])
                xch[j] = t

            def dma_w1(k):
                lo, hi = W1CH[k]
                t = res_pool.tile([128, (hi - lo) * 128], dt.float16,
                                  tag=f"w1_{k}", name=f"w1p{k}")
                nc.scalar.dma_start(t[:], w1_d[:, lo * 128:hi * 128])
                w1p[k] = t

            def dma_gp(p):
                gt = res_pool.tile([128, 2 * NS16 * 128], dt.float16,
                                   tag=f"gp{p}", name=f"gp{p}")
                nc.scalar.dma_start(gt[:], g_d[p])
                gpt[p] = gt

            def dma_g8(p):
                gt = res_pool.tile([128, 2 * K8 * 2, 128], dt.float8e4,
                                   tag=f"g8_{p}", name=f"g8_{p}")
                nc.scalar.dma_start(gt[:], g8_d[p])
                g8t[p] = gt

            def dma_x8(j):
                lo, hi = X8CH[j]
                t = res_pool.tile([128, (hi - lo) * 2, TPC], dt.float8e4,
                                  tag=f"x8c{j}", name=f"x8c{j}")
                nc.scalar.dma_start(t[:], x8_d[:, lo * 2 * TPC:hi * 2 * TPC])
                x8ch[j] = t

            def xslice(i):
                for j, (lo, hi) in enumerate(XCH):
                    if lo <= i < hi:
                        return xch[j][:, (i - lo) * TPC:(i - lo + 1) * TPC]

            def w1slice(slot):
                for k, (lo, hi) in enumerate(W1CH):
                    if lo <= slot < hi:
                        return w1p[k][:, (slot - lo) * 128:(slot - lo + 1) * 128]

            def gslice(g, s):
                off = (g % 2) * NS16 * 128
                return gpt[g // 2][:, off + s * 128:off + (s + 1) * 128]

            def g8slice(g, jj):
                s0 = (g % 2) * 2 * K8 + jj * 2
                return g8t[g // 2][:, s0:s0 + 2, :]

            def x8slice(q):
                for j, (lo, hi) in enumerate(X8CH):
                    if lo <= q < hi:
                        return x8ch[j][:, (q - lo) * 2:(q - lo) * 2 + 2, :]

            # DMA issue order for the stream-in phase (few fat DMAs).
            # Inputs issue on the Activation HWDGE (nc.scalar): its startup
            # prologue clears ~3.5us before SP's, so data flow starts that
            # much earlier; y writes stay on SP's separate 16-queue bank.
            # pos index doubles as the availability ordinal below
            order = ["w1:0", "x:0", "x:1", "g:0", "g8:0", "w1:1", "x:2",
                     "g:1", "g8:1", "x8:0", "x:3", "w1:2", "g:2", "g8:2",
                     "w1:3", "x8:1", "x:4", "x8:2", "x:5", "x8:3"]
            pos = {}
            for p, item in enumerate(order):
                kind, idx = item.split(":")
                {"x": dma_x, "w1": dma_w1, "g": dma_gp, "g8": dma_g8,
                 "x8": dma_x8}[kind](int(idx))
                pos[item] = p
            # bias is only needed at group close (~40us); issue late so the
            # first x/w1 transfers start earlier
            bt = res_pool.tile([128, NG], dt.float32, tag="b")
            nc.scalar.dma_start(bt[:], b_d[:])
            # prefetch all remaining gpack pairs (all-resident, no ring waits)
            for p in range(3, NG // 2):
                dma_gp(p)
                dma_g8(p)

            def xpos(i):
                for j, (lo, hi) in enumerate(XCH):
                    if lo <= i < hi:
                        return pos[f"x:{j}"]

            def w1pos(slot):
                for k, (lo, hi) in enumerate(W1CH):
                    if lo <= slot < hi:
                        return pos[f"w1:{k}"]

            def x8pos(q):
                for j, (lo, hi) in enumerate(X8CH):
                    if lo <= q < hi:
                        return pos[f"x8:{j}"]

            # PE warmup: dummy matmuls on a zeroed tile while the first input
            # DMAs are in flight, so the HAM clock gate reaches 8/8 before
            # the first data matmul. The warm psum reuses the u0 tag: the
            # real u accumulator is the next generation of the same buffer,
            # which orders all warmup matmuls before the first u matmul.
            wt = res_pool.tile([128, TPC], dt.float16, tag="warm")
            nc.vector.memset(wt[:], 0)
            wps = upsum.tile([128, TPC], dt.float32, tag="u0",
                             name="warmps")
            for _ in range(NWARM):
                nc.tensor.matmul(wps[:], wt[:, :128], wt[:],
                                 start=True, stop=True)

            u_ps = [upsum.tile([128, TPC], dt.float32, tag=f"u{lh}",
                               name=f"ups{lh}") for lh in range(2)]

            # merged emission: u matmuls + lead-group butterfly matmuls,
            # sorted by the DMA position that unblocks them
            events = []
            held = []  # last-2 bf per lead: run after last u, hide u_sb cast
            for i in range(NXT):
                av = max(xpos(i), w1pos(i * 2 + 1))
                events.append((av, 0, ("u", i)))
            for g in range(LEADS):
                gav = pos[f"g:{g // 2}"]
                g8av = pos[f"g8:{g // 2}"]
                for s in range(2 * (ACTIVE - K8)):
                    av = max(xpos(int(xtile_idx[g, s])), gav)
                    events.append((av, 1, ("bf", g, s)))
                for jj in range(K8):
                    q = int(xtile_idx[g, (ACTIVE - K8 + jj) * 2]) // 2
                    av = max(x8pos(q), g8av)
                    if jj == K8 - 1:
                        held.append((99, 2, ("dr", g, jj)))
                    else:
                        events.append((av, 1, ("dr", g, jj)))
            events.sort(key=lambda e: (e[0], e[1]))
            events += held
            opened = [False] * NG

            def acc_of(g):
                if accs[g] is None:
                    accs[g] = gpsum.tile([128, TPC], dt.float32,
                                         tag="acc", name=f"acc{g}")
                st, opened[g] = not opened[g], True
                return accs[g], st

            for av, pri, ev in events:
                if ev[0] == "u":
                    i = ev[1]
                    for lh in range(2):
                        nc.tensor.matmul(u_ps[lh][:], w1slice(i * 2 + lh),
                                         xslice(i),
                                         start=(i == 0), stop=(i == NXT - 1))
                elif ev[0] == "bf":
                    _, g, s = ev
                    acc, st = acc_of(g)
                    nc.tensor.matmul(acc[:], gslice(g, s),
                                     xslice(int(xtile_idx[g, s])),
                                     start=st, stop=False)
                else:
                    _, g, jj = ev
                    q = int(xtile_idx[g, (ACTIVE - K8 + jj) * 2]) // 2
                    acc, st = acc_of(g)
                    nc.tensor.matmul(
                        acc[:], g8slice(g, jj), x8slice(q),
                        start=st, stop=False,
                        perf_mode=mybir.MatmulPerfMode.DoubleRowSwInterleave)

            u_sb = []
            for lh in range(2):
                ut = res_pool.tile([128, TPC], dt.float16, tag=f"usb{lh}",
                                   name=f"usb{lh}")
                nc.vector.tensor_copy(ut[:], u_ps[lh][:])
                u_sb.append(ut)

            ych_of = {}
            for ci, (lo, hi) in enumerate(YCH):
                for g in range(lo, hi):
                    ych_of[g] = ci
            ycur = [None]

            def close_group(g):
                for lh in range(2):
                    nc.tensor.matmul(accs[g][:], gslice(g, NS16 - 2 + lh),
                                     u_sb[lh][:],
                                     start=False, stop=(lh == 1))
                ci = ych_of[g]
                lo, hi = YCH[ci]
                if g == lo:
                    ycur[0] = res_pool.tile([128, (hi - lo) * TPC],
                                            dt.float16, tag=f"y{ci}",
                                            name=f"yc{ci}")
                c = g - lo
                if g == NG - 1:
                    # split the final bias-add so the last y DMA issues as
                    # soon as the first half is ready (shorter drain tail)
                    H = TPC // 2
                    for h in range(2):
                        nc.vector.tensor_scalar(
                            ycur[0][:, c * TPC + h * H:c * TPC + (h + 1) * H],
                            accs[g][:, h * H:(h + 1) * H], 1.0 / PSCALE,
                            bt[:, g:g + 1], mybir.AluOpType.mult,
                            mybir.AluOpType.add)
                        nc.sync.dma_start(
                            y_d[:, g * TPC + h * H:g * TPC + (h + 1) * H],
                            ycur[0][:, c * TPC + h * H:c * TPC + (h + 1) * H])
                    return
                nc.vector.tensor_scalar(
                    ycur[0][:, c * TPC:(c + 1) * TPC], accs[g][:],
                    1.0 / PSCALE, bt[:, g:g + 1], mybir.AluOpType.mult,
                    mybir.AluOpType.add)
                if g == hi - 1:
                    nc.sync.dma_start(y_d[:, lo * TPC:hi * TPC], ycur[0][:])

            for g in range(LEADS):
                close_group(g)

            QUAD = 2  # groups whose psum-recycle waits coalesce per batch
            g = LEADS
            while g < NG:
                batch = list(range(g, min(g + QUAD, NG)))
                for gg in batch:  # openers back-to-back: one pipeline break
                    acc, st = acc_of(gg)
                    nc.tensor.matmul(acc[:], gslice(gg, 0),
                                     xslice(int(xtile_idx[gg, 0])),
                                     start=st, stop=False)
                for gg in batch:
                    for s in range(1, 2 * (ACTIVE - K8)):
                        nc.tensor.matmul(accs[gg][:], gslice(gg, s),
                                         xslice(int(xtile_idx[gg, s])),
                                         start=False, stop=False)
                    for jj in range(K8):
                        q = int(xtile_idx[gg, (ACTIVE - K8 + jj) * 2]) // 2
                        nc.tensor.matmul(
                            accs[gg][:], g8slice(gg, jj), x8slice(q),
                            start=False, stop=False,
                            perf_mode=mybir.MatmulPerfMode.DoubleRowSwInterleave)
                    close_group(gg)
                g += QUAD

    nc.compile()
    return nc


def _pack_weights(weight, w1, w2, b, flat):
    import ml_dtypes
    E4 = ml_dtypes.float8_e4m3
    PSCALE = 4096.0
    NBF16 = 2 * (ACTIVE - K8)
    r2 = np.arange(BLOCK)
    gfull = np.empty((NG, 128, 10 * 128), np.float32)  # all bf slots, fp32
    for ob in range(NB):
        for j in range(ACTIVE):
            m = int(flat[ob, j])
            q, a2 = m // ACTIVE, m % ACTIVE
            k = a2 * BLOCK + r2
            Wblk = weight[q * BLOCK + k // ACTIVE, k % ACTIVE, :]  # [r2, c]
            for rh in range(2):
                g = ob * 2 + rh
                for kh in range(2):
                    s = j * 2 + kh
                    gfull[g, :, s * 128:(s + 1) * 128] = \
                        Wblk[rh * 128:(rh + 1) * 128,
                             kh * 128:(kh + 1) * 128].T
    # fp16 part: bf slots j<ACTIVE-K8 and the two w2 slots, scaled x4096
    gpack = np.empty((NG, 128, NS16 * 128), np.float16)
    gpack[:, :, :NBF16 * 128] = gfull[:, :, :NBF16 * 128] * PSCALE
    for g in range(NG):
        for lh in range(2):
            s = NS16 - 2 + lh
            gpack[g, :, s * 128:(s + 1) * 128] = \
                w2[g * 128:(g + 1) * 128,
                   lh * 128:(lh + 1) * 128].T * PSCALE
    gpairs = np.ascontiguousarray(
        gpack.reshape(NG // 2, 2, 128, NS16 * 128)
             .transpose(0, 2, 1, 3)
             .reshape(NG // 2, 128, 2 * NS16 * 128))
    # fp8 part: bf slots j>=ACTIVE-K8, scaled x256, TRN e4m3 clip at 240.
    # DoubleRowSwInterleave layout: per DR pair the two 128-col halves A,B
    # are stored column-interleaved and reversed: A127 B127 A126 ... B0
    g8raw = np.clip(gfull[:, :, NBF16 * 128:] * 256.0, -240, 240).astype(E4)
    g8v = g8raw.reshape(NG, 128, K8, 2, 128)       # [g, p, jj, half, col]
    g8v = g8v[:, :, :, :, ::-1]                    # reverse columns
    g8 = np.ascontiguousarray(
        g8v.transpose(0, 1, 2, 4, 3)               # [g, p, jj, col, half]
    ).reshape(NG, 128, K8 * 2 * 128)
    g8pairs = np.ascontiguousarray(
        g8.reshape(NG // 2, 2, 128, 2 * K8 * 128)
          .transpose(0, 2, 1, 3)
          .reshape(NG // 2, 128, 2 * 2 * K8 * 128))
    w1sb = np.ascontiguousarray(
        w1.reshape(2, 128, 32, 128).transpose(2, 0, 3, 1)
          .reshape(64, 128, 128).transpose(1, 0, 2)
          .reshape(128, 64 * 128)).astype(np.float16)
    bpack = np.ascontiguousarray(b.reshape(NG, 128).T)
    return gpairs, g8pairs, w1sb, bpack


def _ensure_axon_hooks():
    # Some images lack antenv.axon_hooks; bass_utils imports it on the
    # trace path. Provide a stub so trace degrades gracefully.
    import sys
    import types
    try:
        import antenv.axon_hooks  # noqa: F401
        return
    except ImportError:
        pass
    mod = types.ModuleType("antenv.axon_hooks")
    mod._hook = None
    mod.set_axon_ntff_profile_hook = lambda h: setattr(mod, "_hook", h)
    mod.get_axon_ntff_profile_hook = lambda: mod._hook
    sys.modules["antenv.axon_hooks"] = mod
    try:
        import antenv
        antenv.axon_hooks = mod
    except ImportError:
        pass


def kernel(x, weight, w1, w2, b, butterfly_flat_indices):
    _ensure_axon_hooks()
    from concourse.bass_utils import run_bass_kernel_spmd

    x = np.ascontiguousarray(x, np.float32)
    weight = np.ascontiguousarray(weight, np.float32)
    w1 = np.ascontiguousarray(w1, np.float32)
    w2 = np.ascontiguousarray(w2, np.float32)
    b = np.ascontiguousarray(b, np.float32)
    flat = np.asarray(butterfly_flat_indices)

    xtile_idx = _derive_xtile_idx(flat)
    key = xtile_idx.tobytes()
    if key not in _CACHE:
        _CACHE[key] = _build(xtile_idx)
    nc = _CACHE[key]

    import ml_dtypes
    E4 = ml_dtypes.float8_e4m3
    gpairs, g8pairs, w1sb, bpack = _pack_weights(weight, w1, w2, b, flat)
    in_maps = []
    for c in range(NCORES):
        xs = x[c * TPC:(c + 1) * TPC]
        xpf = np.ascontiguousarray(
            xs.T.reshape(NXT, 128, TPC).transpose(1, 0, 2)
              .reshape(128, NXT * TPC))
        xpack = xpf.astype(np.float16)
        x8pack = np.clip(xpf * 16.0, -240, 240).astype(E4)
        in_maps.append({"xpack": xpack, "x8pack": x8pack, "w1pack": w1sb,
                        "gpack": gpairs, "g8pack": g8pairs, "bpack": bpack})

    trace = bool(int(os.environ.get("PIXELFLY_TRACE", "0")))
    res = run_bass_kernel_spmd(nc, in_maps, list(range(NCORES)), trace=trace)
    LAST["exec_time_ns"] = res.exec_time_ns
    LAST["results"] = res

    out = np.empty((TOKENS, OUT_F), np.float32)
    for c in range(NCORES):
        yc = res.results[c]["y"]  # [128, NG*TPC] fp16
        yfull = (yc.reshape(128, NG, TPC).transpose(1, 0, 2)
                   .reshape(OUT_F, TPC))
        out[c * TPC:(c + 1) * TPC] = yfull.T.astype(np.float32)
    return out


# revision 20
# speedup vs baseline: 1.0438x; 1.0438x over previous
import os
import numpy as np

# nn_PixelflyLinear: y = (x @ w1.T) @ w2.T + b + butterfly_matmul(x, weight, flat_idx)
# Data-parallel over tokens: 8 cores x 512 tokens, weights replicated.
# Device computes yT (out_f on partitions, tokens on free dim); host transposes.

TOKENS, IN_F, OUT_F, LOWRANK = 4096, 4096, 4096, 256
BLOCK, ACTIVE, NB = 256, 5, 16
NCORES = 8
TPC = TOKENS // NCORES          # 512 tokens per core
NG = OUT_F // 128               # 32 output half-block groups
NXT = IN_F // 128               # 32 input tiles
NSLOT = 12                      # logical lhsT slots per group (10 bf + 2 lowrank)
K8 = 3                          # butterfly j-slots per group in fp8 DoubleRow
NS16 = 2 * (ACTIVE - K8) + 2    # fp16 slots per group: bf fp16 + 2 w2 slots
PSCALE = 4096.0                 # psum scale: fp16 W x4096; fp8 (Wx256)*(xx16)
NWARM = 7                       # dummy matmuls to trip HAM to 8/8 before real work

_CACHE = {}
LAST = {"exec_time_ns": None}


def _derive_xtile_idx(flat):
    xtile_idx = np.zeros((NG, 10), np.int64)
    for ob in range(NB):
        for j in range(ACTIVE):
            m = int(flat[ob, j])
            q = m // ACTIVE
            for rh in range(2):
                for kh in range(2):
                    xtile_idx[ob * 2 + rh, j * 2 + kh] = q * 2 + kh
    return xtile_idx


def _build(xtile_idx):
    import concourse.bacc as bacc
    import concourse.mybir as mybir
    import concourse.tile as tile

    nc = bacc.Bacc("TRN2", target_bir_lowering=False, debug=False,
                   num_devices=NCORES)
    dt = mybir.dt

    LEADS = 6
    # x-tile chunks (tapered: small first for early PE start, fat later)
    XCH = [(0, 1), (1, 4), (4, 10), (10, 18), (18, 26), (26, 32)]
    # w1 slot ranges per DMA piece (slot = i*2+lh, 64 slots total)
    W1CH = [(0, 4), (4, 12), (12, 32), (32, 64)]
    # y-out group chunks (tapered at the end to shrink the drain tail)
    YCH = [(0, 4), (4, 8), (8, 12), (12, 16), (16, 20), (20, 24), (24, 28),
           (28, 30), (30, 31), (31, 32)]

    xpack_d = nc.dram_tensor("xpack", [128, NXT * TPC], dt.float16,
                             kind="ExternalInput")
    x8_d = nc.dram_tensor("x8pack", [128, NXT * TPC], dt.float8e4,
                          kind="ExternalInput")
    w1_d = nc.dram_tensor("w1pack", [128, 64 * 128], dt.float16,
                          kind="ExternalInput")
    g_d = nc.dram_tensor("gpack", [NG // 2, 128, 2 * NS16 * 128], dt.float16,
                         kind="ExternalInput")
    g8_d = nc.dram_tensor("g8pack", [NG // 2, 128, 2 * K8 * 2 * 128],
                          dt.float8e4, kind="ExternalInput")
    b_d = nc.dram_tensor("bpack", [128, NG], dt.float32, kind="ExternalInput")
    y_d = nc.dram_tensor("y", [128, NG * TPC], dt.float16,
                         kind="ExternalOutput")

    with tile.TileContext(nc) as tc:
        with (
            tc.tile_pool(name="res", bufs=1) as res_pool,
            tc.tile_pool(name="upsum", bufs=1, space="PSUM") as upsum,
            tc.tile_pool(name="gpsum", bufs=6, space="PSUM") as gpsum,
        ):
            X8CH = [(0, 4), (4, 8), (8, 12), (12, 16)]  # block units
            xch = [None] * len(XCH)          # SBUF chunk tiles
            x8ch = [None] * len(X8CH)
            w1p = [None] * len(W1CH)
            gpt = [None] * (NG // 2)         # fp16 gpack pair tiles
            g8t = [None] * (NG // 2)         # fp8 DR pair tiles
            accs = [None] * NG

            def dma_x(j):
                lo, hi = XCH[j]
                t = res_pool.tile([128, (hi - lo) * TPC], dt.float16,
                                  tag=f"xc{j}", name=f"xc{j}")
                nc.scalar.dma_start(t[:], xpack_d[:, lo * TPC:hi * TPC])
                xch[j] = t

            def dma_w1(k):
                lo, hi = W1CH[k]
                t = res_pool.tile([128, (hi - lo) * 128], dt.float16,
                                  tag=f"w1_{k}", name=f"w1p{k}")
                nc.scalar.dma_start(t[:], w1_d[:, lo * 128:hi * 128])
                w1p[k] = t

            def dma_gp(p):
                gt = res_pool.tile([128, 2 * NS16 * 128], dt.float16,
                                   tag=f"gp{p}", name=f"gp{p}")
                nc.scalar.dma_start(gt[:], g_d[p])
                gpt[p] = gt

            def dma_g8(p):
                gt = res_pool.tile([128, 2 * K8 * 2, 128], dt.float8e4,
                                   tag=f"g8_{p}", name=f"g8_{p}")
                nc.scalar.dma_start(gt[:], g8_d[p])
                g8t[p] = gt

            def dma_x8(j):
                lo, hi = X8CH[j]
                t = res_pool.tile([128, (hi - lo) * 2, TPC], dt.float8e4,
                                  tag=f"x8c{j}", name=f"x8c{j}")
                nc.scalar.dma_start(t[:], x8_d[:, lo * 2 * TPC:hi * 2 * TPC])
                x8ch[j] = t

            def xslice(i):
                for j, (lo, hi) in enumerate(XCH):
                    if lo <= i < hi:
                        return xch[j][:, (i - lo) * TPC:(i - lo + 1) * TPC]

            def w1slice(slot):
                for k, (lo, hi) in enumerate(W1CH):
                    if lo <= slot < hi:
                        return w1p[k][:, (slot - lo) * 128:(slot - lo + 1) * 128]

            def gslice(g, s):
                off = (g % 2) * NS16 * 128
                return gpt[g // 2][:, off + s * 128:off + (s + 1) * 128]

            def g8slice(g, jj):
                s0 = (g % 2) * 2 * K8 + jj * 2
                return g8t[g // 2][:, s0:s0 + 2, :]

            def x8slice(q):
                for j, (lo, hi) in enumerate(X8CH):
                    if lo <= q < hi:
                        return x8ch[j][:, (q - lo) * 2:(q - lo) * 2 + 2, :]

            # DMA issue order for the stream-in phase (few fat DMAs).
            # Inputs issue on the Activation HWDGE (nc.scalar): its startup
            # prologue clears ~3.5us before SP's, so data flow starts that
            # much earlier; y writes stay on SP's separate 16-queue bank.
            # pos index doubles as the availability ordinal below
            order = ["w1:0", "x:0", "x:1", "g:0", "g8:0", "w1:1", "x:2",
                     "g:1", "g8:1", "x:3", "w1:2", "g:2", "g8:2", "x8:0",
                     "w1:3", "x:4", "x8:1", "x:5", "x8:2", "x8:3"]
            pos = {}
            for p, item in enumerate(order):
                kind, idx = item.split(":")
                {"x": dma_x, "w1": dma_w1, "g": dma_gp, "g8": dma_g8,
                 "x8": dma_x8}[kind](int(idx))
                pos[item] = p
            # bias is only needed at group close (~40us); issue late so the
            # first x/w1 transfers start earlier
            bt = res_pool.tile([128, NG], dt.float32, tag="b")
            nc.scalar.dma_start(bt[:], b_d[:])
            # prefetch all remaining gpack pairs (all-resident, no ring waits)
            for p in range(3, NG // 2):
                dma_gp(p)
                dma_g8(p)

            def xpos(i):
                for j, (lo, hi) in enumerate(XCH):
                    if lo <= i < hi:
                        return pos[f"x:{j}"]

            def w1pos(slot):
                for k, (lo, hi) in enumerate(W1CH):
                    if lo <= slot < hi:
                        return pos[f"w1:{k}"]

            def x8pos(q):
                for j, (lo, hi) in enumerate(X8CH):
                    if lo <= q < hi:
                        return pos[f"x8:{j}"]

            # PE warmup: dummy matmuls on a zeroed tile while the first input
            # DMAs are in flight, so the HAM clock gate reaches 8/8 before
            # the first data matmul. The warm psum reuses the u0 tag: the
            # real u accumulator is the next generation of the same buffer,
            # which orders all warmup matmuls before the first u matmul.
            wt = res_pool.tile([128, TPC], dt.float16, tag="warm")
            nc.vector.memset(wt[:], 0)
            wps = upsum.tile([128, TPC], dt.float32, tag="u0",
                             name="warmps")
            for _ in range(NWARM):
                nc.tensor.matmul(wps[:], wt[:, :128], wt[:],
                                 start=True, stop=True)

            u_ps = [upsum.tile([128, TPC], dt.float32, tag=f"u{lh}",
                               name=f"ups{lh}") for lh in range(2)]

            # merged emission: u matmuls + lead-group butterfly matmuls,
            # sorted by the DMA position that unblocks them
            events = []
            held = []  # last-2 bf per lead: run after last u, hide u_sb cast
            for i in range(NXT):
                av = max(xpos(i), w1pos(i * 2 + 1))
                events.append((av, 0, ("u", i)))
            for g in range(LEADS):
                gav = pos[f"g:{g // 2}"]
                g8av = pos[f"g8:{g // 2}"]
                for s in range(2 * (ACTIVE - K8)):
                    av = max(xpos(int(xtile_idx[g, s])), gav)
                    events.append((av, 1, ("bf", g, s)))
                for jj in range(K8):
                    q = int(xtile_idx[g, (ACTIVE - K8 + jj) * 2]) // 2
                    av = max(x8pos(q), g8av)
                    if jj == K8 - 1:
                        held.append((99, 2, ("dr", g, jj)))
                    else:
                        events.append((av, 1, ("dr", g, jj)))
            events.sort(key=lambda e: (e[0], e[1]))
            events += held
            opened = [False] * NG

            def acc_of(g):
                if accs[g] is None:
                    accs[g] = gpsum.tile([128, TPC], dt.float32,
                                         tag="acc", name=f"acc{g}")
                st, opened[g] = not opened[g], True
                return accs[g], st

            for av, pri, ev in events:
                if ev[0] == "u":
                    i = ev[1]
                    for lh in range(2):
                        nc.tensor.matmul(u_ps[lh][:], w1slice(i * 2 + lh),
                                         xslice(i),
                                         start=(i == 0), stop=(i == NXT - 1))
                elif ev[0] == "bf":
                    _, g, s = ev
                    acc, st = acc_of(g)
                    nc.tensor.matmul(acc[:], gslice(g, s),
                                     xslice(int(xtile_idx[g, s])),
                                     start=st, stop=False)
                else:
                    _, g, jj = ev
                    q = int(xtile_idx[g, (ACTIVE - K8 + jj) * 2]) // 2
                    acc, st = acc_of(g)
                    nc.tensor.matmul(
                        acc[:], g8slice(g, jj), x8slice(q),
                        start=st, stop=False,
                        perf_mode=mybir.MatmulPerfMode.DoubleRowSwInterleave)

            u_sb = []
            for lh in range(2):
                ut = res_pool.tile([128, TPC], dt.float16, tag=f"usb{lh}",
                                   name=f"usb{lh}")
                nc.vector.tensor_copy(ut[:], u_ps[lh][:])
                u_sb.append(ut)

            ych_of = {}
            for ci, (lo, hi) in enumerate(YCH):
                for g in range(lo, hi):
                    ych_of[g] = ci
            ycur = [None]

            def close_group(g):
                for lh in range(2):
                    nc.tensor.matmul(accs[g][:], gslice(g, NS16 - 2 + lh),
                                     u_sb[lh][:],
                                     start=False, stop=(lh == 1))
                ci = ych_of[g]
                lo, hi = YCH[ci]
                if g == lo:
                    ycur[0] = res_pool.tile([128, (hi - lo) * TPC],
                                            dt.float16, tag=f"y{ci}",
                                            name=f"yc{ci}")
                c = g - lo
                if g == NG - 1:
                    # split the final bias-add so the last y DMA issues as
                    # soon as the first half is ready (shorter drain tail)
                    H = TPC // 2
                    for h in range(2):
                        nc.vector.tensor_scalar(
                            ycur[0][:, c * TPC + h * H:c * TPC + (h + 1) * H],
                            accs[g][:, h * H:(h + 1) * H], 1.0 / PSCALE,
                            bt[:, g:g + 1], mybir.AluOpType.mult,
                            mybir.AluOpType.add)
                        nc.sync.dma_start(
                            y_d[:, g * TPC + h * H:g * TPC + (h + 1) * H],
                            ycur[0][:, c * TPC + h * H:c * TPC + (h + 1) * H])
                    return
                nc.vector.tensor_scalar(
                    ycur[0][:, c * TPC:(c + 1) * TPC], accs[g][:],
                    1.0 / PSCALE, bt[:, g:g + 1], mybir.AluOpType.mult,
                    mybir.AluOpType.add)
                if g == hi - 1:
                    nc.sync.dma_start(y_d[:, lo * TPC:hi * TPC], ycur[0][:])

            for g in range(LEADS):
                close_group(g)

            QUAD = 4  # groups whose psum-recycle waits coalesce per batch
            g = LEADS
            while g < NG:
                batch = list(range(g, min(g + QUAD, NG)))
                for gg in batch:  # openers back-to-back: one pipeline break
                    acc, st = acc_of(gg)
                    nc.tensor.matmul(acc[:], gslice(gg, 0),
                                     xslice(int(xtile_idx[gg, 0])),
                                     start=st, stop=False)
                for gg in batch:
                    for s in range(1, 2 * (ACTIVE - K8)):
                        nc.tensor.matmul(accs[gg][:], gslice(gg, s),
                                         xslice(int(xtile_idx[gg, s])),
                                         start=False, stop=False)
                    for jj in range(K8):
                        q = int(xtile_idx[gg, (ACTIVE - K8 + jj) * 2]) // 2
                        nc.tensor.matmul(
                            accs[gg][:], g8slice(gg, jj), x8slice(q),
                            start=False, stop=False,
                            perf_mode=mybir.MatmulPerfMode.DoubleRowSwInterleave)
                    close_group(gg)
                g += QUAD

    nc.compile()
    return nc


def _pack_weights(weight, w1, w2, b, flat):
    import ml_dtypes
    E4 = ml_dtypes.float8_e4m3
    PSCALE = 4096.0
    NBF16 = 2 * (ACTIVE - K8)
    r2 = np.arange(BLOCK)
    gfull = np.empty((NG, 128, 10 * 128), np.float32)  # all bf slots, fp32
    for ob in range(NB):
        for j in range(ACTIVE):
            m = int(flat[ob, j])
            q, a2 = m // ACTIVE, m % ACTIVE
            k = a2 * BLOCK + r2
            Wblk = weight[q * BLOCK + k // ACTIVE, k % ACTIVE, :]  # [r2, c]
            for rh in range(2):
                g = ob * 2 + rh
                for kh in range(2):
                    s = j * 2 + kh
                    gfull[g, :, s * 128:(s + 1) * 128] = \
                        Wblk[rh * 128:(rh + 1) * 128,
                             kh * 128:(kh + 1) * 128].T
    # fp16 part: bf slots j<ACTIVE-K8 and the two w2 slots, scaled x4096
    gpack = np.empty((NG, 128, NS16 * 128), np.float16)
    gpack[:, :, :NBF16 * 128] = gfull[:, :, :NBF16 * 128] * PSCALE
    for g in range(NG):
        for lh in range(2):
            s = NS16 - 2 + lh
            gpack[g, :, s * 128:(s + 1) * 128] = \
                w2[g * 128:(g + 1) * 128,
                   lh * 128:(lh + 1) * 128].T * PSCALE
    gpairs = np.ascontiguousarray(
        gpack.reshape(NG // 2, 2, 128, NS16 * 128)
             .transpose(0, 2, 1, 3)
             .reshape(NG // 2, 128, 2 * NS16 * 128))
    # fp8 part: bf slots j>=ACTIVE-K8, scaled x256, TRN e4m3 clip at 240.
    # DoubleRowSwInterleave layout: per DR pair the two 128-col halves A,B
    # are stored column-interleaved and reversed: A127 B127 A126 ... B0
    g8raw = np.clip(gfull[:, :, NBF16 * 128:] * 256.0, -240, 240).astype(E4)
    g8v = g8raw.reshape(NG, 128, K8, 2, 128)       # [g, p, jj, half, col]
    g8v = g8v[:, :, :, :, ::-1]                    # reverse columns
    g8 = np.ascontiguousarray(
        g8v.transpose(0, 1, 2, 4, 3)               # [g, p, jj, col, half]
    ).reshape(NG, 128, K8 * 2 * 128)
    g8pairs = np.ascontiguousarray(
        g8.reshape(NG // 2, 2, 128, 2 * K8 * 128)
          .transpose(0, 2, 1, 3)
          .reshape(NG // 2, 128, 2 * 2 * K8 * 128))
    w1sb = np.ascontiguousarray(
        w1.reshape(2, 128, 32, 128).transpose(2, 0, 3, 1)
          .reshape(64, 128, 128).transpose(1, 0, 2)
          .reshape(128, 64 * 128)).astype(np.float16)
    bpack = np.ascontiguousarray(b.reshape(NG, 128).T)
    return gpairs, g8pairs, w1sb, bpack


def _ensure_axon_hooks():
    # Some images lack antenv.axon_hooks; bass_utils imports it on the
    # trace path. Provide a stub so trace degrades gracefully.
    import sys
    import types
    try:
        import antenv.axon_hooks  # noqa: F401
        return
    except ImportError:
        pass
    mod = types.ModuleType("antenv.axon_hooks")
    mod._hook = None
    mod.set_axon_ntff_profile_hook = lambda h: setattr(mod, "_hook", h)
    mod.get_axon_ntff_profile_hook = lambda: mod._hook
    sys.modules["antenv.axon_hooks"] = mod
    try:
        import antenv
        antenv.axon_hooks = mod
    except ImportError:
        pass


def kernel(x, weight, w1, w2, b, butterfly_flat_indices):
    _ensure_axon_hooks()
    from concourse.bass_utils import run_bass_kernel_spmd

    x = np.ascontiguousarray(x, np.float32)
    weight = np.ascontiguousarray(weight, np.float32)
    w1 = np.ascontiguousarray(w1, np.float32)
    w2 = np.ascontiguousarray(w2, np.float32)
    b = np.ascontiguousarray(b, np.float32)
    flat = np.asarray(butterfly_flat_indices)

    xtile_idx = _derive_xtile_idx(flat)
    key = xtile_idx.tobytes()
    if key not in _CACHE:
        _CACHE[key] = _build(xtile_idx)
    nc = _CACHE[key]

    import ml_dtypes
    E4 = ml_dtypes.float8_e4m3
    gpairs, g8pairs, w1sb, bpack = _pack_weights(weight, w1, w2, b, flat)
    in_maps = []
    for c in range(NCORES):
        xs = x[c * TPC:(c + 1) * TPC]
        xpf = np.ascontiguousarray(
            xs.T.reshape(NXT, 128, TPC).transpose(1, 0, 2)
              .reshape(128, NXT * TPC))
        xpack = xpf.astype(np.float16)
        x8pack = np.clip(xpf * 16.0, -240, 240).astype(E4)
        in_maps.append({"xpack": xpack, "x8pack": x8pack, "w1pack": w1sb,
                        "gpack": gpairs, "g8pack": g8pairs, "bpack": bpack})

    trace = bool(int(os.environ.get("PIXELFLY_TRACE", "0")))
    res = run_bass_kernel_spmd(nc, in_maps, list(range(NCORES)), trace=trace)
    LAST["exec_time_ns"] = res.exec_time_ns
    LAST["results"] = res

    out = np.empty((TOKENS, OUT_F), np.float32)
    for c in range(NCORES):
        yc = res.results[c]["y"]  # [128, NG*TPC] fp16
        yfull = (yc.reshape(128, NG, TPC).transpose(1, 0, 2)
                   .reshape(OUT_F, TPC))
        out[c * TPC:(c + 1) * TPC] = yfull.T.astype(np.float32)
    return out
